# revision 68
# baseline (speedup 1.0000x reference)
"""CrossAttention Trainium2 kernel (Bass/Tile), batch-parallel over 8 NeuronCores.

Problem (per batch b of 8):
    x   [512, 32, 32]  -> X   [C=512, N=1024]
    ctx [512, 32, 32]  -> CTX [C=512, M=1024]
    q = Wq@X * s ; k = Wk@CTX ; v = Wv@CTX          (1x1 convs; biases folded)
    per head h (8 heads x 64): simT[j,i] = sum_d k[d,j] q[d,i]
    attn = softmax_j(sim);  out[i,d] = sum_j attn[i,j] v[d,j]
    final = Wo@out + bo

Two hard floors set the shape of this kernel:
  - exp() exists only on the Activation engine (0.833 ns/elem over the free
    dim) and softmax needs 8h*1024i*1024j/128part = 65536 free-elems per
    core: ~58us of exp + per-instruction overhead.
  - the PE charge is out-free-size rows/matmul, so sim (27.3us), the
    projections (q/k/v/o, 27.3us), and attn@v (13.9us) total ~69us in bf16.
    fp8 (DoubleRow) would halve the deep-contraction matmuls but measured
    numerics kill it: the max-rel-err metric is dominated by peaked softmax
    rows, where fp8's 3.6% rms noise on q/k costs 12% (and even es-only or
    v-only fp8 costs ~2%, the whole budget).  So everything stays bf16 and
    PE (~69us) and Act (~66us) are co-critical; the job is keeping both
    streams gapless.

Structure (per core = one batch):
  - sim is computed TRANSPOSED (j on partitions) one 128-j-chunk at a time
    into 2-bank PSUM groups ([128, hb2, 512i]); three 2-bank pools rotate so
    a group's fill is always 2 exp-slots ahead of its drain, keeping the Act
    exp stream gapless (64 exps of [128,1024] back to back).  exp writes
    bf16 es tiles (one per jc pair).
  - attn@v is FLIPPED: lhsT = es[j, i-block], rhs = [v_h | 1], streaming 65
    output rows per matmul; the ones column yields the softmax denominator
    per (i, head).  drain + reciprocal on DVE, normalize on GPSIMD,
    transpose back [i,hd]->[hd,i] with PE identity-matmuls.
  - iterations go pr-major ((pr, ic) pairs of i-512-blocks) so the k/q/vT
    projection jobs spread uniformly; per 8-slot iteration the prev
    iteration's attn@v chains sit at slots 1-4, transposes at 2-5, and jobs
    at slots 0/1 (carried, small) and 5/6 (the boundary slots 7/0 stay
    light so the next iteration's fills are never late for Act).
  - DMA queue assignment matters twice: a dma_start on the scalar queue
    costs 667ns of Act SEQ (which also sequences the exp stream), so bulk
    and output DMAs ride the sync (SP) queue; and HWDGE generation strictly
    alternates between the two queues, which fixes the bus order of the
    prologue-critical wk0 | ctx.h0 | wq0 | x.h0 chain.
  - o-proj: ic0 rows by-oc ([128,512], fat 1KB output descriptors) overlap
    the tail's attn@v chains; ic1 goes per-128-token column behind each
    tail transpose so only the last chunk's chain is on the critical path.
    The tail borrows the idle sim pools so every chain has its own PSUM
    bank.  Output is stored bf16 and the host widens to f32 (~0.1% noise).
  - bias folding (host, exact): bo' = Wo@bv + bo; bk dropped (softmax is
    invariant to per-i shifts); bq via an optional per-head r_h[j] =
    s*bq_h^T(Wk c)_h correction added to sim before exp (compiled only
    when bq != 0).

Host-side prep (NOT device time): weights pre-transposed, everything cast
bf16 and pre-arranged so DMAs are contiguous [128, X] blocks.
"""

import contextlib
import sys

sys.path.insert(0, "/opt/trn_rl_repo")

import numpy as np
import ml_dtypes

import concourse.bass as bass
import concourse.tile as tile
from concourse import bacc, mybir

B = 8
H = 8
DH = 64
C = 512
NTOK = 1024  # 32*32
P = 128
HP = 4  # head pairs
JC = 8  # 128-token j chunks
JP = 4  # j chunk pairs
CCH = 4  # 128-channel contraction chunks
F = 512  # i block
IC2 = 4  # 128-token i sub-chunks per block
SCALE = DH ** (-0.5)

BF16 = mybir.dt.bfloat16
F32 = mybir.dt.float32
NPBF16 = ml_dtypes.bfloat16


def build_nc(reps: int = 1, use_r: bool = False):
    nc = bacc.Bacc("TRN2", target_bir_lowering=False, debug=False)

    x_d = nc.dram_tensor("x", [P, 2, CCH, F], BF16, kind="ExternalInput")
    c_d = nc.dram_tensor("ctx", [P, 2, CCH, F], BF16, kind="ExternalInput")
    wq_d = nc.dram_tensor("wq", [P, HP, CCH, P], BF16, kind="ExternalInput")
    wk_d = nc.dram_tensor("wk", [P, HP, CCH, P], BF16, kind="ExternalInput")
    wv_d = nc.dram_tensor("wv", [P, HP, CCH, P], BF16, kind="ExternalInput")
    wo_d = nc.dram_tensor("wo", [P, CCH, C], BF16, kind="ExternalInput")
    bo2_d = nc.dram_tensor("bo2", [P, CCH], F32, kind="ExternalInput")
    id_d = nc.dram_tensor("ident", [P, P], BF16, kind="ExternalInput")
    if use_r:
        r_d = nc.dram_tensor("r", [P, JC, H], F32, kind="ExternalInput")
    out_d = nc.dram_tensor("out", [C, NTOK], BF16, kind="ExternalOutput")

    with tile.TileContext(nc) as tc:
        with (
            tc.tile_pool(name="consts", bufs=1) as consts,
            tc.tile_pool(name="acts", bufs=1) as acts,
            tc.tile_pool(name="esp", bufs=16) as esp,
            tc.tile_pool(name="attf", bufs=8) as attfp,
            tc.tile_pool(name="attn", bufs=8) as attnp,
            tc.tile_pool(name="recp", bufs=8) as recp,
            tc.tile_pool(name="finp", bufs=10) as finp,
            tc.tile_pool(name="simA", bufs=1, space="PSUM") as simA,
            tc.tile_pool(name="simB", bufs=1, space="PSUM") as simB,
            tc.tile_pool(name="simC", bufs=1, space="PSUM") as simC,
            tc.tile_pool(name="mxps", bufs=2, space="PSUM") as mxps,
        ):
          with (tc.For_i(0, reps, 1) if reps > 1 else contextlib.nullcontext()) as _i:
            # ---- constants ----
            wq_sb = consts.tile([P, HP, CCH, P], BF16, tag="wq")
            wk_sb = consts.tile([P, HP, CCH, P], BF16, tag="wk")
            wv_sb = consts.tile([P, HP, CCH, P], BF16, tag="wv")
            wo_sb = consts.tile([P, CCH, C], BF16, tag="wo")
            bo2_sb = consts.tile([P, CCH], F32, tag="bo2")
            id_sb = consts.tile([P, P], BF16, tag="ident")
            if use_r:
                r_sb = consts.tile([P, JC, H], F32, tag="r")

            x_sb = acts.tile([P, 2, CCH, F], BF16, tag="x")
            c_sb = acts.tile([P, 2, CCH, F], BF16, tag="c")

            # The critical prologue chain is k(0,jh0) [wk.pr0 + ctx.h0] then
            # q(0,ih0) [wq.pr0 + x.h0].  Those four go on ONE queue (sync)
            # in strict order -- split across queues the DMA bus reorders
            # them and the weights land late.  Everything else streams on
            # the scalar queue in consumption order: vT needs wv.pr0 + ctx,
            # k(0,jh1) needs ctx.h1, later pairs need the weight remainder.
            nc.sync.dma_start(out=wk_sb[:, 0], in_=wk_d[:, 0])
            nc.scalar.dma_start(out=c_sb[:, 0], in_=c_d[:, 0])
            nc.sync.dma_start(out=wq_sb[:, 0], in_=wq_d[:, 0])
            nc.scalar.dma_start(out=x_sb[:, 0, :, 0:256], in_=x_d[:, 0, :, 0:256])
            nc.sync.dma_start(out=id_sb[:, :], in_=id_d[:, :])
            nc.scalar.dma_start(out=x_sb[:, 0, :, 256:F], in_=x_d[:, 0, :, 256:F])
            nc.sync.dma_start(out=bo2_sb[:, :], in_=bo2_d[:, :])
            nc.scalar.dma_start(out=c_sb[:, 1, :, 0:256], in_=c_d[:, 1, :, 0:256])
            nc.scalar.dma_start(out=c_sb[:, 1, :, 256:F], in_=c_d[:, 1, :, 256:F])
            nc.sync.dma_start(out=wv_sb[:, 0:1], in_=wv_d[:, 0:1])
            nc.sync.dma_start(out=x_sb[:, 1], in_=x_d[:, 1])
            nc.sync.dma_start(out=wk_sb[:, 1:HP], in_=wk_d[:, 1:HP])
            nc.sync.dma_start(out=wq_sb[:, 1:HP], in_=wq_d[:, 1:HP])
            nc.sync.dma_start(out=wv_sb[:, 1:HP], in_=wv_d[:, 1:HP])
            nc.sync.dma_start(out=wo_sb[:, :, :], in_=wo_d[:, :, :])
            if use_r:
                nc.sync.dma_start(out=r_sb[:, :], in_=r_d[:, :])

            q_sb = acts.tile([P, HP, NTOK], BF16, tag="q")
            k_sb = acts.tile([P, HP, NTOK], BF16, tag="k")
            # transposed v + ones column: [j%128, jc, head, 64+1]
            vte_sb = acts.tile([P, JC, H, DH + 1], BF16, tag="vte")
            oall_sb = acts.tile([P, CCH, NTOK], BF16, tag="oall")

            nc.vector.memset(vte_sb[:, :, :, DH : DH + 1], 1.0)

            # ---- projection banks ----
            def emit_qk(which, pr, ih, nsplit=1, dsplit=False, hi_drain=False):
                """q or k for head pair pr, token half ih: one PSUM bank.
                nsplit=2 emits per-256-token halves so the prologue can
                start on a partially-landed activation DMA."""
                dst, wt, src = (
                    (q_sb, wq_sb, x_sb) if which == "q" else (k_sb, wk_sb, c_sb)
                )
                ps = mxps.tile([P, F], F32, tag="mx", name=f"{which}ps{pr}{ih}")
                w = F // nsplit
                for nn in range(nsplit):
                    for cc in range(CCH):
                        nc.tensor.matmul(
                            ps[:, nn * w : (nn + 1) * w],
                            wt[:, pr, cc, :],
                            src[:, ih, cc, nn * w : (nn + 1) * w],
                            start=(nn == 0 and cc == 0),
                            stop=(nn == nsplit - 1 and cc == CCH - 1),
                            skip_group_check=(nsplit > 1),
                        )
                with (tc.high_priority(offset=3000) if hi_drain
                      else contextlib.nullcontext()):
                    if dsplit:
                        nc.vector.tensor_copy(
                            dst[:, pr, ih * F : ih * F + 256], ps[:, 0:256]
                        )
                        nc.vector.tensor_copy(
                            dst[:, pr, ih * F + 256 : (ih + 1) * F], ps[:, 256:F]
                        )
                    else:
                        nc.vector.tensor_copy(
                            dst[:, pr, ih * F : (ih + 1) * F], ps[:, :]
                        )

            def emit_vt2(pr, mc):
                """vT for head pair pr, token chunks mc and mc+1, sharing one
                PSUM bank (interleaved accumulation groups, one drain)."""
                ps = mxps.tile([P, F], F32, tag="mx", name=f"vps{pr}{mc}")
                for m in (mc, mc + 1):
                    for cc in range(CCH):
                        nc.tensor.matmul(
                            ps[:, (m - mc) * P : (m - mc + 1) * P],
                            c_sb[:, m // 4, cc, (m % 4) * P : (m % 4 + 1) * P],
                            wv_sb[:, pr, cc, :],
                            start=(m == mc and cc == 0),
                            stop=(m == mc + 1 and cc == CCH - 1),
                            skip_group_check=True,
                        )
                nc.vector.tensor_copy(
                    vte_sb[:, mc : mc + 2, 2 * pr : 2 * (pr + 1), 0:DH],
                    ps[:, 0 : 2 * P].rearrange("p (m h d) -> p m h d", m=2, d=DH),
                )

            # ---- PE p-state warmup: throwaway matmuls spanning the initial
            # DMA wait so the real prologue starts at full clock ----
            wu_sb = consts.tile([1, F], BF16, tag="wu")
            nc.vector.memset(wu_sb[:, :], 0.0)

            def job_warm(n):
                def f():
                    ps = mxps.tile([1, F], F32, tag="mx", name="warm")
                    for _ in range(n):
                        nc.tensor.matmul(
                            ps[:, :], wu_sb[:, 0:1], wu_sb[:, :], start=True, stop=True
                        )
                return f

            # ---- attention pieces ----
            def emit_attnv(ps, pes, ppr, ic2):
                """attn@v for one 128-token i sub-chunk, both heads of pair
                ppr: 16 matmuls, one interleaved accumulation group."""
                for jp in range(JP):
                    for tl in range(2):
                        jc = 2 * jp + tl
                        for hb in range(2):
                            nc.tensor.matmul(
                                ps[:, hb * (DH + 1) : (hb + 1) * (DH + 1)],
                                pes[jp][:, hb, tl, ic2 * P : (ic2 + 1) * P],
                                vte_sb[:, jc, 2 * ppr + hb, :],
                                start=(jc == 0 and hb == 0),
                                stop=(jc == JC - 1 and hb == 1),
                                skip_group_check=True,
                            )

            def emit_att_chain(t, pes, ppr, ic2, att_f, att_n, rec, eng=None):
                ps = mxps.tile([P, F], F32, tag="mx", name=f"av{t}{ic2}")
                emit_attnv(ps, pes, ppr, ic2)
                nc.vector.tensor_copy(
                    att_f[:, ic2],
                    ps[:, 0 : 2 * (DH + 1)].rearrange("p (h e) -> p h e", e=DH + 1),
                )
                nc.vector.reciprocal(rec[:, ic2], att_f[:, ic2, :, DH : DH + 1])
                (eng or nc.gpsimd).tensor_tensor(
                    att_n[:, ic2],
                    att_f[:, ic2, :, 0:DH],
                    rec[:, ic2].to_broadcast([P, 2, DH]),
                    mybir.AluOpType.mult,
                )

            def emit_tp(t, pic, ppr, ic2, att_n):
                tps = mxps.tile([P, F], F32, tag="mx", name=f"tp{t}{ic2}")
                tbf = tps[:, 0 : P // 2].bitcast(BF16)
                nc.tensor.transpose(tbf, att_n[:, ic2], id_sb[:, :])
                nc.vector.tensor_copy(
                    oall_sb[:, ppr, pic * F + ic2 * P : pic * F + (ic2 + 1) * P], tbf
                )

            # ---- software-pipelined iteration ----
            pools3 = [simA, simB, simC]

            def emit_oproj(ic, oc, ps=None, on_act=False):
                """o-proj for one 128-row output block, full 512-token
                column: [128, 512] fin rows keep the out-DMA descriptors
                fat (1KB)."""
                if ps is None:
                    ps = mxps.tile([P, F], F32, tag="mx", name=f"op{ic}{oc}")
                for cc in range(CCH):
                    nc.tensor.matmul(
                        ps[:, :],
                        wo_sb[:, cc, oc * P : (oc + 1) * P],
                        oall_sb[:, cc, ic * F : (ic + 1) * F],
                        start=(cc == 0),
                        stop=(cc == CCH - 1),
                    )
                fin = finp.tile([P, F], BF16, tag="fin", name=f"fin{ic}{oc}")
                if on_act:
                    # tail: Act's exp stream is over -- bias-add there so the
                    # DVE queue (drains) isn't the critical path
                    nc.scalar.activation(
                        fin[:, :], ps[:, :],
                        mybir.ActivationFunctionType.Identity,
                        bias=bo2_sb[:, oc : oc + 1],
                    )
                else:
                    nc.vector.tensor_scalar_add(fin[:, :], ps[:, :],
                                                bo2_sb[:, oc : oc + 1])
                nc.sync.dma_start(
                    out=out_d[oc * P : (oc + 1) * P, ic * F : (ic + 1) * F],
                    in_=fin[:, :],
                )

            def emit_iteration(t, ic, pr, es_tiles, prev, jobs):
                """8 sim+exp groups (one jc each, 2 PSUM banks, 3-pool
                rotation: the fill is always 2 exp-slots ahead of Act);
                prev iteration's attn@v / normalize / transpose epilogue and
                streamed projection jobs (slot -> job list) fill the PE
                slack.  Jobs carried to slots 0/1 emit BEFORE that slot's
                attn@v so vte writes always precede their readers."""
                for jc in range(JC):
                    g = pools3[(t * JC + jc) % 3].tile(
                        [P, 2, F], F32, tag="sim", name=f"g{t}{jc}"
                    )
                    for hb in range(2):
                        nc.tensor.matmul(
                            g[:, hb, :],
                            k_sb[hb * DH : (hb + 1) * DH, pr, jc * P : (jc + 1) * P],
                            q_sb[hb * DH : (hb + 1) * DH, pr, ic * F : (ic + 1) * F],
                            start=True,
                            stop=True,
                        )
                    if use_r:
                        for hb in range(2):
                            nc.vector.tensor_scalar_add(
                                g[:, hb, :], g[:, hb, :],
                                r_sb[:, jc, 2 * pr + hb : 2 * pr + hb + 1],
                            )
                    nc.scalar.activation(
                        out=es_tiles[jc // 2][:, :, jc % 2, :],
                        in_=g[:, :, :],
                        func=mybir.ActivationFunctionType.Exp,
                    )
                    if jc <= 1:
                        for job in jobs.get(jc, []):
                            job()
                    if prev is not None:
                        pic, ppr, pes, patt, pattn, prec = prev
                        if 2 <= jc <= 5:
                            emit_att_chain(t - 1, pes, ppr, jc - 2, patt, pattn, prec)
                        if 3 <= jc <= 6:
                            emit_tp(t - 1, pic, ppr, jc - 3, pattn)
                    if jc >= 2:
                        for job in jobs.get(jc, []):
                            job()

            def job_qk(which, pr, ih, dsplit=False, hi=False):
                return lambda: emit_qk(which, pr, ih, dsplit=dsplit, hi_drain=hi)

            def job_vt(pr, mc):
                return lambda: emit_vt2(pr, mc)

            # prologue: a few warmup matmuls to start the PE p-state ramp
            # while the first DMAs land, then the critical k/q banks
            job_warm(3)()
            emit_qk("k", 0, 0)
            emit_qk("q", 0, 0, nsplit=2)
            emit_qk("k", 0, 1, nsplit=2, dsplit=True)

            iters = [(pr, ic) for pr in range(HP) for ic in range(2)]
            JOBS = {
                0: {2: [job_vt(0, 0)], 3: [job_vt(0, 2)],
                    4: [job_vt(0, 4)], 5: [job_qk("q", 0, 1)], 6: [job_vt(0, 6)]},
                1: {0: [job_vt(1, 0)], 5: [job_qk("q", 1, 0)],
                    6: [job_qk("k", 1, 0, dsplit=True)]},
                2: {0: [job_qk("k", 1, 1)], 1: [job_vt(1, 2)],
                    5: [job_qk("q", 1, 1)], 6: [job_vt(1, 4)]},
                3: {0: [job_vt(1, 6)], 5: [job_qk("q", 2, 0)],
                    6: [job_qk("k", 2, 0, dsplit=True)]},
                4: {0: [job_qk("k", 2, 1)], 1: [job_vt(2, 0)],
                    4: [job_vt(2, 4)],
                    5: [job_qk("q", 2, 1)], 6: [job_vt(2, 2)]},
                5: {0: [job_vt(2, 6)], 5: [job_qk("q", 3, 0)],
                    6: [job_qk("k", 3, 0, dsplit=True)]},
                6: {0: [job_qk("k", 3, 1)], 1: [job_vt(3, 0)],
                    4: [job_vt(3, 4)],
                    5: [job_qk("q", 3, 1)], 6: [job_vt(3, 2)]},
                7: {0: [job_vt(3, 6)]},
            }
            prev = None
            for t, (pr, ic) in enumerate(iters):
                es_tiles = [
                    esp.tile([P, 2, 2, F], BF16, tag="es", name=f"es{t}j{jp}")
                    for jp in range(JP)
                ]
                att_f = attfp.tile([P, IC2, 2, DH + 1], F32, tag="attf", name=f"af{t}")
                att_n = attnp.tile([P, IC2, 2, DH], BF16, tag="attn", name=f"an{t}")
                rec = recp.tile([P, IC2, 2, 1], F32, tag="rec", name=f"rc{t}")
                emit_iteration(t, ic, pr, es_tiles, prev, JOBS[t])
                prev = (ic, pr, es_tiles, att_f, att_n, rec)

            # tail: remaining ic0 o-proj rows, the last iteration's attn@v
            # chunks back-to-back, transposes, then ic1 o-proj rows.  The
            # sim pools are free here, so every chain gets its own PSUM
            # bank instead of strangling through the 2 rotating mxps bufs.
            TL = len(iters) - 1
            pic, ppr, pes, patt, pattn, prec = prev
            tailp = [simA, simB, simC, mxps]

            def tail_ps(i, nm):
                # sim pools keep their [P, 2, F] tile shape (pool sizing is
                # per-shape); callers use bank 0 of the 2-bank tile
                if i % 4 < 3:
                    return tailp[i % 4].tile([P, 2, F], F32, tag="sim",
                                             name=f"{nm}{i}")[:, 0]
                return tailp[3].tile([P, F], F32, tag="mx", name=f"{nm}{i}")

            emit_oproj(0, 0)
            emit_oproj(0, 1, on_act=True)
            for ic2 in range(IC2):
                ps = tail_ps(ic2, "avT")
                emit_attnv(ps, pes, ppr, ic2)
                nc.vector.tensor_copy(
                    patt[:, ic2],
                    ps[:, 0 : 2 * (DH + 1)].rearrange("p (h e) -> p h e", e=DH + 1),
                )
                nc.vector.reciprocal(prec[:, ic2], patt[:, ic2, :, DH : DH + 1])
                (nc.gpsimd if ic2 < 3 else nc.vector).tensor_tensor(
                    pattn[:, ic2],
                    patt[:, ic2, :, 0:DH],
                    prec[:, ic2].to_broadcast([P, 2, DH]),
                    mybir.AluOpType.mult,
                )
                if ic2 == 0:
                    emit_oproj(0, 2)
                if ic2 == 1:
                    emit_oproj(0, 3, on_act=True)
            for ic2 in range(IC2):
                emit_tp(TL, pic, ppr, ic2, pattn)
            for ic2 in range(IC2):
                base = pic * F + ic2 * P
                ps = tail_ps(ic2, "opT")
                for cc in range(CCH):
                    for oc in range(CCH):
                        nc.tensor.matmul(
                            ps[:, oc * P : (oc + 1) * P],
                            wo_sb[:, cc, oc * P : (oc + 1) * P],
                            oall_sb[:, cc, base : base + P],
                            start=(cc == 0 and oc == 0),
                            stop=(cc == CCH - 1 and oc == CCH - 1),
                            skip_group_check=True,
                        )
                fin = finp.tile([P, CCH, P], BF16, tag="finc", name=f"finT{ic2}")
                nc.vector.tensor_tensor(
                    fin[:, :, :],
                    ps.rearrange("p (a n) -> p a n", a=CCH),
                    bo2_sb.to_broadcast([P, CCH, P]),
                    mybir.AluOpType.add,
                )
                nc.sync.dma_start(
                    out=out_d[:, base : base + P].rearrange("(a p) n -> p a n", p=P),
                    in_=fin[:, :, :],
                )

    nc.compile()
    return nc


def prep_inputs(x, context, Wq, bq, Wk, bk, Wv, bv, Wo, bo):
    """Host-side sharding + layout prep. Returns per-core input maps."""
    xb = np.asarray(x, np.float32).reshape(B, C, NTOK)
    cb = np.asarray(context, np.float32).reshape(B, C, NTOK)

    def lay_act(v):  # [C, NTOK] -> [P, NH2, CCH, F]: contiguous token halves
        return np.ascontiguousarray(
            v.reshape(CCH, P, 2, F).transpose(1, 2, 0, 3)
        ).astype(NPBF16)

    def lay_w(wl):  # [C_in, C_out] -> [P, HP(out), CCH(in), 128(out)]
        return np.ascontiguousarray(
            wl.reshape(CCH, P, HP, P).transpose(1, 2, 0, 3)
        ).astype(NPBF16)

    wq16 = lay_w(np.asarray(Wq, np.float32).T * SCALE)
    wk16 = lay_w(np.asarray(Wk, np.float32).T)
    wv16 = lay_w(np.asarray(Wv, np.float32).T)
    wo16 = np.ascontiguousarray(
        np.asarray(Wo, np.float32).T.reshape(CCH, P, C).transpose(1, 0, 2)
    ).astype(NPBF16)
    # exact bias folding: out = Wo@(att + bv*1) + bo = Wo@att + (Wo@bv + bo);
    # bk is dropped (softmax is invariant to adding a constant per query i)
    bo2 = np.ascontiguousarray(
        (np.asarray(Wo, np.float32) @ np.asarray(bv, np.float32)
         + np.asarray(bo, np.float32)).reshape(CCH, P).T
    ).astype(np.float32)
    ident = np.eye(P, dtype=NPBF16)
    bqf = np.asarray(bq, np.float32)
    use_r = bool(np.any(bqf))
    in_maps = []
    for b in range(B):
        m = {
            "x": lay_act(xb[b]),
            "ctx": lay_act(cb[b]),
            "wq": wq16,
            "wk": wk16,
            "wv": wv16,
            "wo": wo16,
            "bo2": bo2,
            "ident": ident,
        }
        if use_r:
            # r_h[j] = s * bq_h^T (Wk @ ctx_b)_h[:, j], the bq-dependent sim
            # term, per head (each head contracts only its own 64 channels)
            kb = np.asarray(Wk, np.float32) @ cb[b]
            rh = SCALE * np.einsum(
                "hd,hdj->hj", bqf.reshape(H, DH), kb.reshape(H, DH, NTOK)
            )
            m["r"] = np.ascontiguousarray(
                rh.reshape(H, JC, P).transpose(2, 1, 0).astype(np.float32)
            )
        in_maps.append(m)
    return in_maps


_NC = {}


def _get_nc(use_r: bool = False):
    if use_r not in _NC:
        _NC[use_r] = build_nc(use_r=use_r)
    return _NC[use_r]


def kernel(x, context, Wq, bq, Wk, bk, Wv, bv, Wo, bo):
    from concourse.bass_utils import run_bass_kernel_spmd

    in_maps = prep_inputs(x, context, Wq, bq, Wk, bk, Wv, bv, Wo, bo)
    nc = _get_nc("r" in in_maps[0])
    br = run_bass_kernel_spmd(nc, in_maps, list(range(B)))
    out = np.stack(
        [np.asarray(br.results[b]["out"]).astype(np.float32) for b in range(B)]
    )
    return out.reshape(B, C, 32, 32)


# revision 69
# speedup vs baseline: 1.0009x; 1.0009x over previous
"""CrossAttention Trainium2 kernel (Bass/Tile), batch-parallel over 8 NeuronCores.

Problem (per batch b of 8):
    x   [512, 32, 32]  -> X   [C=512, N=1024]
    ctx [512, 32, 32]  -> CTX [C=512, M=1024]
    q = Wq@X * s ; k = Wk@CTX ; v = Wv@CTX          (1x1 convs; biases folded)
    per head h (8 heads x 64): simT[j,i] = sum_d k[d,j] q[d,i]
    attn = softmax_j(sim);  out[i,d] = sum_j attn[i,j] v[d,j]
    final = Wo@out + bo

Two hard floors set the shape of this kernel:
  - exp() exists only on the Activation engine (0.833 ns/elem over the free
    dim) and softmax needs 8h*1024i*1024j/128part = 65536 free-elems per
    core: ~58us of exp + per-instruction overhead.
  - the PE charge is out-free-size rows/matmul, so sim (27.3us), the
    projections (q/k/v/o, 27.3us), and attn@v (13.9us) total ~69us in bf16.
    fp8 (DoubleRow) would halve the deep-contraction matmuls but measured
    numerics kill it: the max-rel-err metric is dominated by peaked softmax
    rows, where fp8's 3.6% rms noise on q/k costs 12% (and even es-only or
    v-only fp8 costs ~2%, the whole budget).  So everything stays bf16 and
    PE (~69us) and Act (~66us) are co-critical; the job is keeping both
    streams gapless.

Structure (per core = one batch):
  - sim is computed TRANSPOSED (j on partitions) one 128-j-chunk at a time
    into 2-bank PSUM groups ([128, hb2, 512i]); three 2-bank pools rotate so
    a group's fill is always 2 exp-slots ahead of its drain, keeping the Act
    exp stream gapless (64 exps of [128,1024] back to back).  exp writes
    bf16 es tiles (one per jc pair).
  - attn@v is FLIPPED: lhsT = es[j, i-block], rhs = [v_h | 1], streaming 65
    output rows per matmul; the ones column yields the softmax denominator
    per (i, head).  drain + reciprocal on DVE, normalize on GPSIMD,
    transpose back [i,hd]->[hd,i] with PE identity-matmuls.
  - iterations go pr-major ((pr, ic) pairs of i-512-blocks) so the k/q/vT
    projection jobs spread uniformly; per 8-slot iteration the prev
    iteration's attn@v chains sit at slots 1-4, transposes at 2-5, and jobs
    at slots 0/1 (carried, small) and 5/6 (the boundary slots 7/0 stay
    light so the next iteration's fills are never late for Act).
  - DMA queue assignment matters twice: a dma_start on the scalar queue
    costs 667ns of Act SEQ (which also sequences the exp stream), so bulk
    and output DMAs ride the sync (SP) queue; and HWDGE generation strictly
    alternates between the two queues, which fixes the bus order of the
    prologue-critical wk0 | ctx.h0 | wq0 | x.h0 chain.
  - o-proj: ic0 rows by-oc ([128,512], fat 1KB output descriptors) overlap
    the tail's attn@v chains; ic1 goes per-128-token column behind each
    tail transpose so only the last chunk's chain is on the critical path.
    The tail borrows the idle sim pools so every chain has its own PSUM
    bank.  Output is stored bf16 and the host widens to f32 (~0.1% noise).
  - bias folding (host, exact): bo' = Wo@bv + bo; bk dropped (softmax is
    invariant to per-i shifts); bq via an optional per-head r_h[j] =
    s*bq_h^T(Wk c)_h correction added to sim before exp (compiled only
    when bq != 0).

Host-side prep (NOT device time): weights pre-transposed, everything cast
bf16 and pre-arranged so DMAs are contiguous [128, X] blocks.
"""

import contextlib
import sys

sys.path.insert(0, "/opt/trn_rl_repo")

import numpy as np
import ml_dtypes

import concourse.bass as bass
import concourse.tile as tile
from concourse import bacc, mybir

B = 8
H = 8
DH = 64
C = 512
NTOK = 1024  # 32*32
P = 128
HP = 4  # head pairs
JC = 8  # 128-token j chunks
JP = 4  # j chunk pairs
CCH = 4  # 128-channel contraction chunks
F = 512  # i block
IC2 = 4  # 128-token i sub-chunks per block
SCALE = DH ** (-0.5)

BF16 = mybir.dt.bfloat16
F32 = mybir.dt.float32
NPBF16 = ml_dtypes.bfloat16


def build_nc(reps: int = 1, use_r: bool = False):
    nc = bacc.Bacc("TRN2", target_bir_lowering=False, debug=False)

    x_d = nc.dram_tensor("x", [P, 2, CCH, F], BF16, kind="ExternalInput")
    c_d = nc.dram_tensor("ctx", [P, 2, CCH, F], BF16, kind="ExternalInput")
    wq_d = nc.dram_tensor("wq", [P, HP, CCH, P], BF16, kind="ExternalInput")
    wk_d = nc.dram_tensor("wk", [P, HP, CCH, P], BF16, kind="ExternalInput")
    wv_d = nc.dram_tensor("wv", [P, HP, CCH, P], BF16, kind="ExternalInput")
    wo_d = nc.dram_tensor("wo", [P, CCH, C], BF16, kind="ExternalInput")
    bo2_d = nc.dram_tensor("bo2", [P, CCH], F32, kind="ExternalInput")
    id_d = nc.dram_tensor("ident", [P, P], BF16, kind="ExternalInput")
    if use_r:
        r_d = nc.dram_tensor("r", [P, JC, H], F32, kind="ExternalInput")
    out_d = nc.dram_tensor("out", [C, NTOK], BF16, kind="ExternalOutput")

    with tile.TileContext(nc) as tc:
        with (
            tc.tile_pool(name="consts", bufs=1) as consts,
            tc.tile_pool(name="acts", bufs=1) as acts,
            tc.tile_pool(name="esp", bufs=16) as esp,
            tc.tile_pool(name="attf", bufs=8) as attfp,
            tc.tile_pool(name="attn", bufs=8) as attnp,
            tc.tile_pool(name="recp", bufs=8) as recp,
            tc.tile_pool(name="finp", bufs=10) as finp,
            tc.tile_pool(name="simA", bufs=1, space="PSUM") as simA,
            tc.tile_pool(name="simB", bufs=1, space="PSUM") as simB,
            tc.tile_pool(name="simC", bufs=1, space="PSUM") as simC,
            tc.tile_pool(name="mxps", bufs=2, space="PSUM") as mxps,
        ):
          with (tc.For_i(0, reps, 1) if reps > 1 else contextlib.nullcontext()) as _i:
            # ---- constants ----
            wq_sb = consts.tile([P, HP, CCH, P], BF16, tag="wq")
            wk_sb = consts.tile([P, HP, CCH, P], BF16, tag="wk")
            wv_sb = consts.tile([P, HP, CCH, P], BF16, tag="wv")
            wo_sb = consts.tile([P, CCH, C], BF16, tag="wo")
            bo2_sb = consts.tile([P, CCH], F32, tag="bo2")
            id_sb = consts.tile([P, P], BF16, tag="ident")
            if use_r:
                r_sb = consts.tile([P, JC, H], F32, tag="r")

            x_sb = acts.tile([P, 2, CCH, F], BF16, tag="x")
            c_sb = acts.tile([P, 2, CCH, F], BF16, tag="c")

            # The critical prologue chain is k(0,jh0) [wk.pr0 + ctx.h0] then
            # q(0,ih0) [wq.pr0 + x.h0].  Those four go on ONE queue (sync)
            # in strict order -- split across queues the DMA bus reorders
            # them and the weights land late.  Everything else streams on
            # the scalar queue in consumption order: vT needs wv.pr0 + ctx,
            # k(0,jh1) needs ctx.h1, later pairs need the weight remainder.
            nc.sync.dma_start(out=wk_sb[:, 0], in_=wk_d[:, 0])
            nc.scalar.dma_start(out=c_sb[:, 0], in_=c_d[:, 0])
            nc.sync.dma_start(out=wq_sb[:, 0], in_=wq_d[:, 0])
            nc.scalar.dma_start(out=x_sb[:, 0, :, 0:256], in_=x_d[:, 0, :, 0:256])
            nc.sync.dma_start(out=id_sb[:, :], in_=id_d[:, :])
            nc.scalar.dma_start(out=x_sb[:, 0, :, 256:F], in_=x_d[:, 0, :, 256:F])
            nc.sync.dma_start(out=bo2_sb[:, :], in_=bo2_d[:, :])
            nc.scalar.dma_start(out=c_sb[:, 1, :, 0:256], in_=c_d[:, 1, :, 0:256])
            nc.scalar.dma_start(out=c_sb[:, 1, :, 256:F], in_=c_d[:, 1, :, 256:F])
            nc.sync.dma_start(out=wv_sb[:, 0:1], in_=wv_d[:, 0:1])
            nc.sync.dma_start(out=x_sb[:, 1], in_=x_d[:, 1])
            nc.sync.dma_start(out=wk_sb[:, 1:HP], in_=wk_d[:, 1:HP])
            nc.sync.dma_start(out=wq_sb[:, 1:HP], in_=wq_d[:, 1:HP])
            nc.sync.dma_start(out=wv_sb[:, 1:HP], in_=wv_d[:, 1:HP])
            nc.sync.dma_start(out=wo_sb[:, :, :], in_=wo_d[:, :, :])
            if use_r:
                nc.sync.dma_start(out=r_sb[:, :], in_=r_d[:, :])

            q_sb = acts.tile([P, HP, NTOK], BF16, tag="q")
            k_sb = acts.tile([P, HP, NTOK], BF16, tag="k")
            # transposed v + ones column: [j%128, jc, head, 64+1]
            vte_sb = acts.tile([P, JC, H, DH + 1], BF16, tag="vte")
            oall_sb = acts.tile([P, CCH, NTOK], BF16, tag="oall")

            nc.vector.memset(vte_sb[:, :, :, DH : DH + 1], 1.0)

            # ---- projection banks ----
            def emit_qk(which, pr, ih, nsplit=1, dsplit=False, hi_drain=False):
                """q or k for head pair pr, token half ih: one PSUM bank.
                nsplit=2 emits per-256-token halves so the prologue can
                start on a partially-landed activation DMA."""
                dst, wt, src = (
                    (q_sb, wq_sb, x_sb) if which == "q" else (k_sb, wk_sb, c_sb)
                )
                ps = mxps.tile([P, F], F32, tag="mx", name=f"{which}ps{pr}{ih}")
                w = F // nsplit
                for nn in range(nsplit):
                    for cc in range(CCH):
                        nc.tensor.matmul(
                            ps[:, nn * w : (nn + 1) * w],
                            wt[:, pr, cc, :],
                            src[:, ih, cc, nn * w : (nn + 1) * w],
                            start=(nn == 0 and cc == 0),
                            stop=(nn == nsplit - 1 and cc == CCH - 1),
                            skip_group_check=(nsplit > 1),
                        )
                with (tc.high_priority(offset=3000) if hi_drain
                      else contextlib.nullcontext()):
                    if dsplit:
                        nc.vector.tensor_copy(
                            dst[:, pr, ih * F : ih * F + 256], ps[:, 0:256]
                        )
                        nc.vector.tensor_copy(
                            dst[:, pr, ih * F + 256 : (ih + 1) * F], ps[:, 256:F]
                        )
                    else:
                        nc.vector.tensor_copy(
                            dst[:, pr, ih * F : (ih + 1) * F], ps[:, :]
                        )

            def emit_vt2(pr, mc):
                """vT for head pair pr, token chunks mc and mc+1, sharing one
                PSUM bank (interleaved accumulation groups, one drain)."""
                ps = mxps.tile([P, F], F32, tag="mx", name=f"vps{pr}{mc}")
                for m in (mc, mc + 1):
                    for cc in range(CCH):
                        nc.tensor.matmul(
                            ps[:, (m - mc) * P : (m - mc + 1) * P],
                            c_sb[:, m // 4, cc, (m % 4) * P : (m % 4 + 1) * P],
                            wv_sb[:, pr, cc, :],
                            start=(m == mc and cc == 0),
                            stop=(m == mc + 1 and cc == CCH - 1),
                            skip_group_check=True,
                        )
                nc.vector.tensor_copy(
                    vte_sb[:, mc : mc + 2, 2 * pr : 2 * (pr + 1), 0:DH],
                    ps[:, 0 : 2 * P].rearrange("p (m h d) -> p m h d", m=2, d=DH),
                )

            # ---- PE p-state warmup: throwaway matmuls spanning the initial
            # DMA wait so the real prologue starts at full clock ----
            wu_sb = consts.tile([1, F], BF16, tag="wu")
            nc.vector.memset(wu_sb[:, :], 0.0)

            def job_warm(n):
                def f():
                    ps = mxps.tile([1, F], F32, tag="mx", name="warm")
                    for _ in range(n):
                        nc.tensor.matmul(
                            ps[:, :], wu_sb[:, 0:1], wu_sb[:, :], start=True, stop=True
                        )
                return f

            # ---- attention pieces ----
            def emit_attnv(ps, pes, ppr, ic2):
                """attn@v for one 128-token i sub-chunk, both heads of pair
                ppr: 16 matmuls, one interleaved accumulation group."""
                for jp in range(JP):
                    for tl in range(2):
                        jc = 2 * jp + tl
                        for hb in range(2):
                            nc.tensor.matmul(
                                ps[:, hb * (DH + 1) : (hb + 1) * (DH + 1)],
                                pes[jp][:, hb, tl, ic2 * P : (ic2 + 1) * P],
                                vte_sb[:, jc, 2 * ppr + hb, :],
                                start=(jc == 0 and hb == 0),
                                stop=(jc == JC - 1 and hb == 1),
                                skip_group_check=True,
                            )

            def emit_att_chain(t, pes, ppr, ic2, att_f, att_n, rec, eng=None):
                ps = mxps.tile([P, F], F32, tag="mx", name=f"av{t}{ic2}")
                emit_attnv(ps, pes, ppr, ic2)
                nc.vector.tensor_copy(
                    att_f[:, ic2],
                    ps[:, 0 : 2 * (DH + 1)].rearrange("p (h e) -> p h e", e=DH + 1),
                )
                nc.vector.reciprocal(rec[:, ic2], att_f[:, ic2, :, DH : DH + 1])
                (eng or nc.gpsimd).tensor_tensor(
                    att_n[:, ic2],
                    att_f[:, ic2, :, 0:DH],
                    rec[:, ic2].to_broadcast([P, 2, DH]),
                    mybir.AluOpType.mult,
                )

            def emit_tp(t, pic, ppr, ic2, att_n):
                tps = mxps.tile([P, F], F32, tag="mx", name=f"tp{t}{ic2}")
                tbf = tps[:, 0 : P // 2].bitcast(BF16)
                nc.tensor.transpose(tbf, att_n[:, ic2], id_sb[:, :])
                nc.vector.tensor_copy(
                    oall_sb[:, ppr, pic * F + ic2 * P : pic * F + (ic2 + 1) * P], tbf
                )

            # ---- software-pipelined iteration ----
            pools3 = [simA, simB, simC]

            def emit_oproj(ic, oc, ps=None, on_act=False):
                """o-proj for one 128-row output block, full 512-token
                column: [128, 512] fin rows keep the out-DMA descriptors
                fat (1KB)."""
                if ps is None:
                    ps = mxps.tile([P, F], F32, tag="mx", name=f"op{ic}{oc}")
                for cc in range(CCH):
                    nc.tensor.matmul(
                        ps[:, :],
                        wo_sb[:, cc, oc * P : (oc + 1) * P],
                        oall_sb[:, cc, ic * F : (ic + 1) * F],
                        start=(cc == 0),
                        stop=(cc == CCH - 1),
                    )
                fin = finp.tile([P, F], BF16, tag="fin", name=f"fin{ic}{oc}")
                if on_act:
                    # tail: Act's exp stream is over -- bias-add there so the
                    # DVE queue (drains) isn't the critical path
                    nc.scalar.activation(
                        fin[:, :], ps[:, :],
                        mybir.ActivationFunctionType.Identity,
                        bias=bo2_sb[:, oc : oc + 1],
                    )
                else:
                    nc.vector.tensor_scalar_add(fin[:, :], ps[:, :],
                                                bo2_sb[:, oc : oc + 1])
                nc.sync.dma_start(
                    out=out_d[oc * P : (oc + 1) * P, ic * F : (ic + 1) * F],
                    in_=fin[:, :],
                )

            def emit_iteration(t, ic, pr, es_tiles, prev, jobs):
                """8 sim+exp groups (one jc each, 2 PSUM banks, 3-pool
                rotation: the fill is always 2 exp-slots ahead of Act);
                prev iteration's attn@v / normalize / transpose epilogue and
                streamed projection jobs (slot -> job list) fill the PE
                slack.  Jobs carried to slots 0/1 emit BEFORE that slot's
                attn@v so vte writes always precede their readers."""
                for jc in range(JC):
                    g = pools3[(t * JC + jc) % 3].tile(
                        [P, 2, F], F32, tag="sim", name=f"g{t}{jc}"
                    )
                    for hb in range(2):
                        nc.tensor.matmul(
                            g[:, hb, :],
                            k_sb[hb * DH : (hb + 1) * DH, pr, jc * P : (jc + 1) * P],
                            q_sb[hb * DH : (hb + 1) * DH, pr, ic * F : (ic + 1) * F],
                            start=True,
                            stop=True,
                        )
                    if use_r:
                        for hb in range(2):
                            nc.vector.tensor_scalar_add(
                                g[:, hb, :], g[:, hb, :],
                                r_sb[:, jc, 2 * pr + hb : 2 * pr + hb + 1],
                            )
                    nc.scalar.activation(
                        out=es_tiles[jc // 2][:, :, jc % 2, :],
                        in_=g[:, :, :],
                        func=mybir.ActivationFunctionType.Exp,
                    )
                    if jc <= 1:
                        for job in jobs.get(jc, []):
                            job()
                    if prev is not None:
                        pic, ppr, pes, patt, pattn, prec = prev
                        if 2 <= jc <= 5:
                            emit_att_chain(t - 1, pes, ppr, jc - 2, patt, pattn, prec)
                        if 3 <= jc <= 6:
                            emit_tp(t - 1, pic, ppr, jc - 3, pattn)
                    if jc >= 2:
                        for job in jobs.get(jc, []):
                            job()

            def job_qk(which, pr, ih, dsplit=False, hi=False):
                return lambda: emit_qk(which, pr, ih, dsplit=dsplit, hi_drain=hi)

            def job_vt(pr, mc):
                return lambda: emit_vt2(pr, mc)

            # prologue: a few warmup matmuls to start the PE p-state ramp
            # while the first DMAs land, then the critical k/q banks
            job_warm(3)()
            emit_qk("k", 0, 0)
            emit_qk("q", 0, 0, nsplit=2)
            emit_qk("k", 0, 1, nsplit=2, dsplit=True)

            iters = [(pr, ic) for pr in range(HP) for ic in range(2)]
            JOBS = {
                0: {2: [job_vt(0, 0)], 3: [job_vt(0, 2)],
                    4: [job_vt(0, 4)], 5: [job_vt(0, 6)], 6: [job_qk("q", 0, 1)]},
                1: {0: [job_vt(1, 0)], 5: [job_qk("q", 1, 0)],
                    6: [job_qk("k", 1, 0, dsplit=True)]},
                2: {0: [job_qk("k", 1, 1)], 1: [job_vt(1, 2)],
                    5: [job_qk("q", 1, 1)], 6: [job_vt(1, 4)]},
                3: {0: [job_vt(1, 6)], 5: [job_qk("q", 2, 0)],
                    6: [job_qk("k", 2, 0, dsplit=True)]},
                4: {0: [job_qk("k", 2, 1)], 1: [job_vt(2, 0)],
                    4: [job_vt(2, 4)],
                    5: [job_qk("q", 2, 1)], 6: [job_vt(2, 2)]},
                5: {0: [job_vt(2, 6)], 5: [job_qk("q", 3, 0)],
                    6: [job_qk("k", 3, 0, dsplit=True)]},
                6: {0: [job_qk("k", 3, 1)], 1: [job_vt(3, 0)],
                    4: [job_vt(3, 4)],
                    5: [job_qk("q", 3, 1)], 6: [job_vt(3, 2)]},
                7: {0: [job_vt(3, 6)]},
            }
            prev = None
            for t, (pr, ic) in enumerate(iters):
                es_tiles = [
                    esp.tile([P, 2, 2, F], BF16, tag="es", name=f"es{t}j{jp}")
                    for jp in range(JP)
                ]
                att_f = attfp.tile([P, IC2, 2, DH + 1], F32, tag="attf", name=f"af{t}")
                att_n = attnp.tile([P, IC2, 2, DH], BF16, tag="attn", name=f"an{t}")
                rec = recp.tile([P, IC2, 2, 1], F32, tag="rec", name=f"rc{t}")
                emit_iteration(t, ic, pr, es_tiles, prev, JOBS[t])
                prev = (ic, pr, es_tiles, att_f, att_n, rec)

            # tail: remaining ic0 o-proj rows, the last iteration's attn@v
            # chunks back-to-back, transposes, then ic1 o-proj rows.  The
            # sim pools are free here, so every chain gets its own PSUM
            # bank instead of strangling through the 2 rotating mxps bufs.
            TL = len(iters) - 1
            pic, ppr, pes, patt, pattn, prec = prev
            tailp = [simA, simB, simC, mxps]

            def tail_ps(i, nm):
                # sim pools keep their [P, 2, F] tile shape (pool sizing is
                # per-shape); callers use bank 0 of the 2-bank tile
                if i % 4 < 3:
                    return tailp[i % 4].tile([P, 2, F], F32, tag="sim",
                                             name=f"{nm}{i}")[:, 0]
                return tailp[3].tile([P, F], F32, tag="mx", name=f"{nm}{i}")

            emit_oproj(0, 0)
            emit_oproj(0, 1, on_act=True)
            for ic2 in range(IC2):
                ps = tail_ps(ic2, "avT")
                emit_attnv(ps, pes, ppr, ic2)
                nc.vector.tensor_copy(
                    patt[:, ic2],
                    ps[:, 0 : 2 * (DH + 1)].rearrange("p (h e) -> p h e", e=DH + 1),
                )
                nc.vector.reciprocal(prec[:, ic2], patt[:, ic2, :, DH : DH + 1])
                (nc.gpsimd if ic2 < 3 else nc.vector).tensor_tensor(
                    pattn[:, ic2],
                    patt[:, ic2, :, 0:DH],
                    prec[:, ic2].to_broadcast([P, 2, DH]),
                    mybir.AluOpType.mult,
                )
                if ic2 == 0:
                    emit_oproj(0, 2)
                if ic2 == 1:
                    emit_oproj(0, 3, on_act=True)
            for ic2 in range(IC2):
                emit_tp(TL, pic, ppr, ic2, pattn)
            for ic2 in range(IC2):
                base = pic * F + ic2 * P
                ps = tail_ps(ic2, "opT")
                for cc in range(CCH):
                    for oc in range(CCH):
                        nc.tensor.matmul(
                            ps[:, oc * P : (oc + 1) * P],
                            wo_sb[:, cc, oc * P : (oc + 1) * P],
                            oall_sb[:, cc, base : base + P],
                            start=(cc == 0 and oc == 0),
                            stop=(cc == CCH - 1 and oc == CCH - 1),
                            skip_group_check=True,
                        )
                fin = finp.tile([P, CCH, P], BF16, tag="finc", name=f"finT{ic2}")
                nc.vector.tensor_tensor(
                    fin[:, :, :],
                    ps.rearrange("p (a n) -> p a n", a=CCH),
                    bo2_sb.to_broadcast([P, CCH, P]),
                    mybir.AluOpType.add,
                )
                nc.sync.dma_start(
                    out=out_d[:, base : base + P].rearrange("(a p) n -> p a n", p=P),
                    in_=fin[:, :, :],
                )

    nc.compile()
    return nc


def prep_inputs(x, context, Wq, bq, Wk, bk, Wv, bv, Wo, bo):
    """Host-side sharding + layout prep. Returns per-core input maps."""
    xb = np.asarray(x, np.float32).reshape(B, C, NTOK)
    cb = np.asarray(context, np.float32).reshape(B, C, NTOK)

    def lay_act(v):  # [C, NTOK] -> [P, NH2, CCH, F]: contiguous token halves
        return np.ascontiguousarray(
            v.reshape(CCH, P, 2, F).transpose(1, 2, 0, 3)
        ).astype(NPBF16)

    def lay_w(wl):  # [C_in, C_out] -> [P, HP(out), CCH(in), 128(out)]
        return np.ascontiguousarray(
            wl.reshape(CCH, P, HP, P).transpose(1, 2, 0, 3)
        ).astype(NPBF16)

    wq16 = lay_w(np.asarray(Wq, np.float32).T * SCALE)
    wk16 = lay_w(np.asarray(Wk, np.float32).T)
    wv16 = lay_w(np.asarray(Wv, np.float32).T)
    wo16 = np.ascontiguousarray(
        np.asarray(Wo, np.float32).T.reshape(CCH, P, C).transpose(1, 0, 2)
    ).astype(NPBF16)
    # exact bias folding: out = Wo@(att + bv*1) + bo = Wo@att + (Wo@bv + bo);
    # bk is dropped (softmax is invariant to adding a constant per query i)
    bo2 = np.ascontiguousarray(
        (np.asarray(Wo, np.float32) @ np.asarray(bv, np.float32)
         + np.asarray(bo, np.float32)).reshape(CCH, P).T
    ).astype(np.float32)
    ident = np.eye(P, dtype=NPBF16)
    bqf = np.asarray(bq, np.float32)
    use_r = bool(np.any(bqf))
    in_maps = []
    for b in range(B):
        m = {
            "x": lay_act(xb[b]),
            "ctx": lay_act(cb[b]),
            "wq": wq16,
            "wk": wk16,
            "wv": wv16,
            "wo": wo16,
            "bo2": bo2,
            "ident": ident,
        }
        if use_r:
            # r_h[j] = s * bq_h^T (Wk @ ctx_b)_h[:, j], the bq-dependent sim
            # term, per head (each head contracts only its own 64 channels)
            kb = np.asarray(Wk, np.float32) @ cb[b]
            rh = SCALE * np.einsum(
                "hd,hdj->hj", bqf.reshape(H, DH), kb.reshape(H, DH, NTOK)
            )
            m["r"] = np.ascontiguousarray(
                rh.reshape(H, JC, P).transpose(2, 1, 0).astype(np.float32)
            )
        in_maps.append(m)
    return in_maps


_NC = {}


def _get_nc(use_r: bool = False):
    if use_r not in _NC:
        _NC[use_r] = build_nc(use_r=use_r)
    return _NC[use_r]


def kernel(x, context, Wq, bq, Wk, bk, Wv, bv, Wo, bo):
    from concourse.bass_utils import run_bass_kernel_spmd

    in_maps = prep_inputs(x, context, Wq, bq, Wk, bk, Wv, bv, Wo, bo)
    nc = _get_nc("r" in in_maps[0])
    br = run_bass_kernel_spmd(nc, in_maps, list(range(B)))
    out = np.stack(
        [np.asarray(br.results[b]["out"]).astype(np.float32) for b in range(B)]
    )
    return out.reshape(B, C, 32, 32)
